# revision 34
# baseline (speedup 1.0000x reference)
"""Distributed causal self-attention kernel for one TRN2 chip (8 NeuronCores).

Problem: B=2, T=2048, C=1024, H=16 heads, D=64. f32 in/out.
Measured: ~261 us NEFF exec time, rel err (fro) ~3.8e-3 vs the fp32 reference.

Sharding: DP=2 over batch x TP=4 over heads.
  core c -> (b = c//4, g = c%4), owns heads 4g..4g+3 of batch b.

Per-core device program (SPMD, identical graph on all 8 cores), built with
Tile and scheduled as one fused stream so the PE never idles long enough for
the HAM clock gate to re-throttle:

  - qk^T = (x @ [W_q/8 | W_k])^T computed directly in transposed layout via
    matmul(lhsT=W_qk_tile, rhs=x^T_tile); x^T is fed pre-transposed from the
    host, so NO on-chip transposes are needed anywhere. float32r inputs
    (full-rate fp32 PE path), fp32 PSUM, bf16 evacuation fused with the
    per-partition q/k bias add.
  - v = x @ W_v in natural layout, head-interleaved with a ones column
    -> lhsT = [v_h | 1] so the attention AV matmul also produces the softmax
    row-sums for free (row 64 of the accumulator).
  - attention per 512-token chunk, heads in pairs: the even head's q/k rows
    sit at partitions 0-63 and the odd head's at 64-127, so interleaved S^T
    matmuls (K=64, bf16) alternate PE row groups and their weight loads
    overlap in-flight matmuls. exp on ScalarE ([128,1024] PSUM->SBUF, bf16
    out, softmax scale pre-folded into W_q on the host); causal masking of
    diagonal tiles via precomputed multiplicative bf16 masks on VectorE
    (keeps GpSimd free so collective triggers fire promptly). No max-
    subtraction is needed: S = qk/sqrt(D) is O(5) here, exp is safe in fp32.
    Normalization: rowsum broadcast across partitions (GpSimd), fast
    reciprocal + multiply on VectorE, bf16 y^T.
  - cross-core reduction: per-chunk 8-core AllGather of bf16 y^T chunks
    (chunks 0+1 merged; a tiny warmup AllGather at kernel start hides the
    ncfw cold-start). Output rows are rank-major = [batch0 | batch1]
    channels, so every core reads identical offsets (SPMD-safe); the host
    zero-pads each core's W_proj column-slice to 2048 rows so the projection
    contraction picks out its own batch.
  - projection out^chunk = y_gath^T.T @ W_proj_pad (bf16), interleaved into
    later attention chunks' head pairs to fill ACT-bound PE bubbles and to
    guarantee each gather has completed before its matmuls issue (the PE
    stream is in-order).

Host: shards inputs (x transposed per batch, W_attn column-sliced with the
softmax scale folded into W_q, W_proj column-sliced + batch-zero-padded),
reassembles the 8 [2048, 256] output column-slices, adds b_proj (exact for
the final linear step).
"""

import numpy as np

import concourse.bass as bass
import concourse.bacc as bacc
import concourse.mybir as mybir
import concourse.tile as tile
from concourse import bass_utils

F32 = mybir.dt.float32
F32R = mybir.dt.float32r
BF16 = mybir.dt.bfloat16

B, T, C = 2, 2048, 1024
H, D = 16, 64
DP, TP = 2, 4
HPC = H // TP            # 4 heads per core
CH = HPC * D             # 256 channels per core
NCORES = DP * TP

RG8 = [[0, 1, 2, 3, 4, 5, 6, 7]]


def build_kernel(trace_sim: bool = False):
    nc = bacc.Bacc("TRN2", target_bir_lowering=False, debug=False,
                   num_devices=NCORES)

    x_t = nc.dram_tensor("x_t", [C, T], F32R, kind="ExternalInput").ap()
    w_qk = nc.dram_tensor("w_qk", [C, 2 * CH], F32R, kind="ExternalInput").ap()
    b_qk = nc.dram_tensor("b_qk", [2 * CH], F32, kind="ExternalInput").ap()
    w_v = nc.dram_tensor("w_v", [C, CH], F32R, kind="ExternalInput").ap()
    b_v = nc.dram_tensor("b_v", [CH], F32, kind="ExternalInput").ap()
    w_p = nc.dram_tensor("w_p", [B * C, CH], F32, kind="ExternalInput").ap()
    out = nc.dram_tensor("out", [T, CH], F32, kind="ExternalOutput").ap()

    KT = C // 128        # 8 contraction tiles for C
    KT2 = B * C // 128   # 16 contraction tiles for the padded projection
    NTT = T // 128       # 16 token tiles
    NTC = T // 512       # 4 token chunks

    from contextlib import ExitStack
    with tile.TileContext(nc, trace_sim=trace_sim) as tc, ExitStack() as ctx:
        const = ctx.enter_context(tc.tile_pool(name="const", bufs=1))
        qkp = ctx.enter_context(tc.tile_pool(name="qkp", bufs=1))
        vp = ctx.enter_context(tc.tile_pool(name="vp", bufs=1))
        yp = ctx.enter_context(tc.tile_pool(name="yp", bufs=1))
        ep = ctx.enter_context(tc.tile_pool(name="ep", bufs=4))
        rp = ctx.enter_context(tc.tile_pool(name="rp", bufs=2))
        rbp = ctx.enter_context(tc.tile_pool(name="rbp", bufs=2))
        wpp = ctx.enter_context(tc.tile_pool(name="wpp", bufs=1))
        yfp = ctx.enter_context(tc.tile_pool(name="yfp", bufs=2))
        osb = ctx.enter_context(tc.tile_pool(name="osb", bufs=3))
        dram = ctx.enter_context(tc.tile_pool(name="dram", bufs=1, space="DRAM"))

        # ---- persistent SBUF tensors + loads -------------------------------
        Wqk = [const.tile([128, 2 * CH], F32R, name=f"wqk{k}") for k in range(KT)]
        Wv = [const.tile([128, CH], F32R, name=f"wv{k}") for k in range(KT)]
        bqk = const.tile([128, 4], F32, name="bqk")
        bvrow = const.tile([1, CH], F32, name="bvrow")
        bvbc = const.tile([128, CH], F32, name="bvbc")
        ones4 = const.tile([128, 4], F32, name="ones4")
        nc.vector.memset(ones4[:], 1.0)
        # causal masks for the two diagonal s-tile-pair positions, applied
        # multiplicatively on DVE (keeps GpSimd free so AllGather triggers
        # are never queued behind mask work)
        dmask = [const.tile([128, 1024], BF16, name=f"dmask{r}") for r in range(2)]
        for r in range(2):
            nc.gpsimd.memset(dmask[r][:], 1.0)
            nc.gpsimd.affine_select(
                out=dmask[r][:], in_=dmask[r][:],
                compare_op=mybir.AluOpType.is_ge, fill=0.0,
                base=-256 * r, pattern=[[-128, 2], [1, 512]],
                channel_multiplier=-1)

        cc_win = dram.tile([8, 16], BF16, name="cc_win")
        cc_wout = dram.tile([64, 16], BF16, name="cc_wout", addr_space="Shared")
        warm_sb = const.tile([8, 16], BF16, name="warm_sb")
        nc.vector.memset(warm_sb[:], 0.0)
        # dummy exp so the ~2.7us ACT table load happens during the DMA phase
        act_warm = const.tile([1, 16], F32, name="act_warm")
        nc.vector.memset(act_warm[:], 0.0)
        nc.scalar.activation(act_warm[:], act_warm[:],
                             mybir.ActivationFunctionType.Exp)
        nc.sync.dma_start(cc_win[:], warm_sb[:])
        nc.gpsimd.collective_compute(
            "AllGather", mybir.AluOpType.bypass, replica_groups=RG8,
            ins=[cc_win.opt()], outs=[cc_wout.opt()])
        for k in range(KT):
            nc.scalar.dma_start(Wqk[k][:], w_qk[128 * k:128 * k + 128, :])
        nc.sync.dma_start(bqk[:], b_qk.rearrange("(i p) -> p i", p=128))
        nc.sync.dma_start(bvrow[:], b_v.unsqueeze(0))
        nc.gpsimd.partition_broadcast(bvbc[:], bvrow[:])

        # W_proj (padded to 2048 rows) -> bf16 tiles, converted on device.
        # (tiles declared here; DMAs emitted after the x loads below so the
        # first QKV matmuls aren't starved behind 2MB of projection weights)
        Wp = [wpp.tile([128, CH], BF16, name=f"wp{k}") for k in range(KT2)]
        wpf = [wpp.tile([128, CH], F32, name=f"wpf{k}") for k in range(KT2)]

        # qk^T tiles (bf16): [o-tile i][t-chunk j] -> [128, 512]
        # i = 0,1: q rows (pre-scaled by 1/sqrt(D) on host); i = 2,3: k rows
        qkT = [[qkp.tile([128, 512], BF16, name=f"qkT{i}_{j}") for j in range(NTC)]
               for i in range(4)]
        # v tiles (bf16), head-interleaved with a ones column: [128, 4*65]
        v_sb = [vp.tile([128, HPC * (D + 1)], BF16, name=f"v{m}") for m in range(NTT)]
        # normalized y^T chunk tiles (bf16): [chunk j] -> [256, 512] as 2x128
        yT = [[yp.tile([128, 512], BF16, name=f"yT{i}_{j}") for j in range(NTC)]
              for i in range(2)]

        # tiny warmup AllGather: pays the ncfw cold-start cost (~11us) during
        # the QKV phase so the first real AllGather begins promptly

        # AllGather bounce buffers: chunks 0+1 ship together (halves the
        # serialized collective count early on), chunks 2 and 3 ship alone
        cc_w = [1024, 512, 512]      # token width per ship s
        cc_in = [dram.tile([CH, w], BF16, name=f"cc_in{s}")
                 for s, w in enumerate(cc_w)]
        cc_out = [dram.tile([NCORES * CH, w], BF16, name=f"cc_out{s}",
                            addr_space="Shared")
                  for s, w in enumerate(cc_w)]

        xp = ctx.enter_context(tc.tile_pool(name="xp", bufs=1))
        # x^T loaded in 512-column chunks; chunk 0 lands right after Wqk so
        # the first QKV matmul group starts as early as possible
        xT = [[xp.tile([128, 512], F32R, name=f"xT{k}_{j}") for j in range(NTC)]
              for k in range(KT)]
        for k in range(KT):
            nc.sync.dma_start(xT[k][0][:],
                              x_t[128 * k:128 * k + 128, 0:512])
        for k in range(KT):
            nc.scalar.dma_start(Wv[k][:], w_v[128 * k:128 * k + 128, :])
        for j in range(1, NTC):
            for k in range(KT):
                nc.sync.dma_start(xT[k][j][:],
                                  x_t[128 * k:128 * k + 128,
                                      512 * j:512 * j + 512])
        for k in range(KT2):
            nc.sync.dma_start(wpf[k][:], w_p[128 * k:128 * k + 128, :])
            nc.vector.tensor_copy(Wp[k][:], wpf[k][:])

        # ---- phases C/D/E: chunk-major attention + pipelined AG + proj -----
        def qk_group(j):
            # qk^T = W_qk^T @ x^T for one token chunk
            for i in range(4):
                ps = psM.tile([128, 512], F32, name="psA", tag="psM")
                for k in range(KT):
                    nc.tensor.matmul(
                        ps[:],
                        Wqk[k][:, 128 * i:128 * i + 128],
                        xT[k][j][:],
                        start=(k == 0), stop=(k == KT - 1))
                nc.vector.tensor_scalar_add(qkT[i][j][:], ps[:], bqk[:, i:i + 1])

        def v_group(j):
            # v = x @ W_v (natural layout, +bias, head-interleaved + ones col)
            for m in range(4 * j, 4 * j + 4):
                ones_ap = v_sb[m].rearrange("p (h x) -> p h x", x=D + 1)[:, :, D:D + 1]
                nc.vector.tensor_copy(ones_ap, ones4.rearrange("p (h x) -> p h x", x=1))
                ps = psM.tile([128, CH], F32, name="psB", tag="psM")
                for k in range(KT):
                    nc.tensor.matmul(
                        ps[:],
                        xT[k][m // 4][:, 128 * (m % 4):128 * (m % 4) + 128],
                        Wv[k][:],
                        start=(k == 0), stop=(k == KT - 1))
                v_ap = v_sb[m].rearrange("p (h x) -> p h x", x=D + 1)[:, :, 0:D]
                nc.vector.tensor_add(
                    v_ap,
                    ps.rearrange("p (h d) -> p h d", d=D),
                    bvbc.rearrange("p (h d) -> p h d", d=D))

        # Heads are processed in pairs (2hp, 2hp+1). The even head's q/k rows
        # live at partitions 0-63, the odd head's at 64-127, so interleaving
        # their S matmuls alternates PE row groups (tile_position auto-derives
        # from base_partition): the next weight load overlaps the in-flight
        # matmul and the two K=64 matmuls stream concurrently.
        def attn_chunk(j):
            for hp in range(HPC // 2):
                attn_pair(j, hp)

        def attn_pair(j, hp):
                ha, hb = 2 * hp, 2 * hp + 1
                y_psA = psY.tile([D + 1, 512], F32, name="y_psA", tag="y_ps")
                y_psB = psY.tile([D + 1, 512], F32, name="y_psB", tag="y_ps")
                n_s = 4 * (j + 1)           # causal s-tiles for this chunk
                for sp in range(n_s // 2):  # pairs of 128-row s-tiles
                    sA = psS.tile([128, 1024], F32, name="sA", tag="s_ps")
                    sB = psS.tile([128, 1024], F32, name="sB", tag="s_ps")
                    eA = ep.tile([128, 1024], BF16, name="eA", tag="e_sb")
                    eB = ep.tile([128, 1024], BF16, name="eB", tag="e_sb")
                    for half in range(2):
                        st = 2 * sp + half
                        kt = qkT[2 + hp][st // 4]
                        qt = qkT[hp][j]
                        ks = 128 * (st % 4)
                        nc.tensor.matmul(
                            sA[:, 512 * half:512 * half + 512],
                            kt[0:64, ks:ks + 128], qt[0:64, :],
                            start=True, stop=True)
                        nc.tensor.matmul(
                            sB[:, 512 * half:512 * half + 512],
                            kt[64:128, ks:ks + 128], qt[64:128, :],
                            start=True, stop=True)
                    nc.scalar.activation(
                        eA[:], sA[:], mybir.ActivationFunctionType.Exp)
                    nc.scalar.activation(
                        eB[:], sB[:], mybir.ActivationFunctionType.Exp)
                    if 2 * sp >= 4 * j:     # pair straddles the diagonal
                        r_idx = (2 * sp - 4 * j) // 2
                        for e in (eA, eB):
                            nc.vector.tensor_mul(e[:], e[:], dmask[r_idx][:])
                    for half in range(2):
                        st = 2 * sp + half
                        nc.tensor.matmul(
                            y_psA[:],
                            v_sb[st][:, (D + 1) * ha:(D + 1) * ha + D + 1],
                            eA[:, 512 * half:512 * half + 512],
                            start=(st == 0), stop=(st == n_s - 1))
                        nc.tensor.matmul(
                            y_psB[:],
                            v_sb[st][:, (D + 1) * hb:(D + 1) * hb + D + 1],
                            eB[:, 512 * half:512 * half + 512],
                            start=(st == 0), stop=(st == n_s - 1))
                # normalize: y * (1/rowsum); broadcast the raw rowsum across
                # partitions first so the reciprocal runs at full width
                for hh, y_ps in ((ha, y_psA), (hb, y_psB)):
                    r_sb = rp.tile([1, 512], F32, name="r_sb", tag="r_sb")
                    nc.vector.tensor_copy(r_sb[:], y_ps[D:D + 1, :])
                    rbc = rbp.tile([D, 512], F32, name="rbc", tag="rbc")
                    rinv = rbp.tile([D, 512], F32, name="rinv", tag="rinv")
                    nc.gpsimd.partition_broadcast(rbc[:], r_sb[:])
                    nc.vector.reciprocal_approx_fast(rinv[:], rbc[:])
                    nc.vector.tensor_mul(
                        yT[hp][j][64 * (hh % 2):64 * (hh % 2) + 64, :],
                        y_ps[0:D, :], rinv[:])

        def ship(s, chunks):
            for i in range(2):
                for ci, j in enumerate(chunks):
                    nc.sync.dma_start(
                        cc_in[s][128 * i:128 * i + 128,
                                 512 * ci:512 * ci + 512],
                        yT[i][j][:])
            nc.gpsimd.collective_compute(
                "AllGather", mybir.AluOpType.bypass,
                replica_groups=RG8,
                ins=[cc_in[s].opt()], outs=[cc_out[s].opt()])

        def load_yf(s, col0=0):
            # loads a 512-token column window of ship s's gathered buffer
            yf = [yfp.tile([128, 512], BF16, name=f"yf{k}", tag=f"yf{k}")
                  for k in range(KT2)]
            for k in range(KT2):
                nc.sync.dma_start(yf[k][:],
                                  cc_out[s][128 * k:128 * k + 128,
                                            col0:col0 + 512])
            return yf

        def proj_part(tok0, yf, ms):
            # ms indexes 128-token tiles within this ship's gathered width
            for m in ms:
                o_sb = osb.tile([128, CH], F32, name="o_sb", tag="o_sb")
                ps = psM.tile([128, CH], F32, name="psE", tag="psM")
                for k in range(KT2):
                    nc.tensor.matmul(
                        ps[:],
                        yf[k][:, 128 * m:128 * m + 128],
                        Wp[k][:],
                        start=(k == 0), stop=(k == KT2 - 1))
                nc.vector.tensor_copy(o_sb[:], ps[:])
                nc.scalar.dma_start(
                    out[tok0 + 128 * m:tok0 + 128 * m + 128, :],
                    o_sb[:])

        # QKV chunk-groups feed directly into their attention chunks: the
        # dense QKV/proj matmuls interleave with the ACT-bound attention so
        # the PE never idles long enough for HAM to re-throttle. Chunks 0+1
        # gather together; their projection rides inside attention chunk 3.
        with tc.tile_pool(name="psS", bufs=2, space="PSUM") as psS, \
             tc.tile_pool(name="psY", bufs=2, space="PSUM") as psY, \
             tc.tile_pool(name="psM", bufs=2, space="PSUM") as psM:
            qk_group(0)
            v_group(0)
            attn_chunk(0)
            qk_group(1)
            v_group(1)
            attn_chunk(1)
            ship(0, [0, 1])
            qk_group(2)
            v_group(2)
            attn_chunk(2)
            ship(1, [2])
            qk_group(3)
            v_group(3)
            yf0a = load_yf(0, 0)
            attn_pair(3, 0)
            proj_part(0, yf0a, [0, 1, 2, 3])
            yf0b = load_yf(0, 512)
            attn_pair(3, 1)
            proj_part(512, yf0b, [0])
            ship(2, [3])
            proj_part(1024, load_yf(1), [0, 1, 2, 3])
            # held-back ready work: fills the PE while the last AllGather
            # (chunk 3) is still in flight
            proj_part(512, yf0b, [1, 2, 3])
            proj_part(1536, load_yf(2), [0, 1, 2, 3])

    nc.compile()
    return nc


def shard_inputs(x, W_attn, b_attn, W_proj, b_proj):
    scale = np.float32(D ** -0.5)
    in_maps = []
    for c in range(NCORES):
        b, g = divmod(c, TP)
        q = slice(CH * g, CH * (g + 1))
        k = slice(C + CH * g, C + CH * (g + 1))
        v = slice(2 * C + CH * g, 2 * C + CH * (g + 1))
        W_qk = np.concatenate([W_attn[:, q] * scale, W_attn[:, k]], axis=1)
        b_qk = np.concatenate([b_attn[q] * scale, b_attn[k]])
        # W_proj column slice, zero-padded to 2048 rows so the contraction
        # over the 8-core-gathered [2048, t] y picks out this core's batch
        w_p_pad = np.zeros((B * C, CH), dtype=np.float32)
        w_p_pad[C * b:C * (b + 1)] = W_proj[:, CH * g:CH * (g + 1)]
        in_maps.append({
            "x_t": np.ascontiguousarray(x[b].T, dtype=np.float32),
            "w_qk": np.ascontiguousarray(W_qk, dtype=np.float32),
            "b_qk": np.ascontiguousarray(b_qk, dtype=np.float32),
            "w_v": np.ascontiguousarray(W_attn[:, v], dtype=np.float32),
            "b_v": np.ascontiguousarray(b_attn[v], dtype=np.float32),
            "w_p": w_p_pad,
        })
    return in_maps


_NC_CACHE = {}


def get_compiled():
    if "nc" not in _NC_CACHE:
        _NC_CACHE["nc"] = build_kernel()
    return _NC_CACHE["nc"]


def run_on_hw(in_maps, **kwargs):
    nc = get_compiled()
    return bass_utils.run_bass_kernel_spmd(
        nc, in_maps, core_ids=list(range(NCORES)), **kwargs)


def kernel(x, W_attn, b_attn, W_proj, b_proj):
    x = np.asarray(x, dtype=np.float32)
    W_attn = np.asarray(W_attn, dtype=np.float32)
    b_attn = np.asarray(b_attn, dtype=np.float32)
    W_proj = np.asarray(W_proj, dtype=np.float32)
    b_proj = np.asarray(b_proj, dtype=np.float32)

    in_maps = shard_inputs(x, W_attn, b_attn, W_proj, b_proj)
    res = run_on_hw(in_maps)
    out = np.zeros((B, T, C), dtype=np.float32)
    for c in range(NCORES):
        b, g = divmod(c, TP)
        out[b, :, CH * g:CH * (g + 1)] = res.results[c]["out"]
    out += b_proj[None, None, :]
    return out


# revision 35
# speedup vs baseline: 1.0767x; 1.0767x over previous
"""Distributed causal self-attention kernel for one TRN2 chip (8 NeuronCores).

Problem: B=2, T=2048, C=1024, H=16 heads, D=64. f32 in/out.
Measured: ~261 us NEFF exec time, rel err (fro) ~3.8e-3 vs the fp32 reference.

Sharding: DP=2 over batch x TP=4 over heads.
  core c -> (b = c//4, g = c%4), owns heads 4g..4g+3 of batch b.

Per-core device program (SPMD, identical graph on all 8 cores), built with
Tile and scheduled as one fused stream so the PE never idles long enough for
the HAM clock gate to re-throttle:

  - qk^T = (x @ [W_q/8 | W_k])^T computed directly in transposed layout via
    matmul(lhsT=W_qk_tile, rhs=x^T_tile); x^T is fed pre-transposed from the
    host, so NO on-chip transposes are needed anywhere. float32r inputs
    (full-rate fp32 PE path), fp32 PSUM, bf16 evacuation fused with the
    per-partition q/k bias add.
  - v = x @ W_v in natural layout, head-interleaved with a ones column
    -> lhsT = [v_h | 1] so the attention AV matmul also produces the softmax
    row-sums for free (row 64 of the accumulator).
  - attention per 512-token chunk, heads in pairs: the even head's q/k rows
    sit at partitions 0-63 and the odd head's at 64-127, so interleaved S^T
    matmuls (K=64, bf16) alternate PE row groups and their weight loads
    overlap in-flight matmuls. exp on ScalarE ([128,1024] PSUM->SBUF, bf16
    out, softmax scale pre-folded into W_q on the host); causal masking of
    diagonal tiles via precomputed multiplicative bf16 masks on VectorE
    (keeps GpSimd free so collective triggers fire promptly). No max-
    subtraction is needed: S = qk/sqrt(D) is O(5) here, exp is safe in fp32.
    Normalization: rowsum broadcast across partitions (GpSimd), fast
    reciprocal + multiply on VectorE, bf16 y^T.
  - cross-core reduction: per-chunk 8-core AllGather of bf16 y^T chunks
    (chunks 0+1 merged; a tiny warmup AllGather at kernel start hides the
    ncfw cold-start). Output rows are rank-major = [batch0 | batch1]
    channels, so every core reads identical offsets (SPMD-safe); the host
    zero-pads each core's W_proj column-slice to 2048 rows so the projection
    contraction picks out its own batch.
  - projection out^chunk = y_gath^T.T @ W_proj_pad (bf16), interleaved into
    later attention chunks' head pairs to fill ACT-bound PE bubbles and to
    guarantee each gather has completed before its matmuls issue (the PE
    stream is in-order).

Host: shards inputs (x transposed per batch, W_attn column-sliced with the
softmax scale folded into W_q, W_proj column-sliced + batch-zero-padded),
reassembles the 8 [2048, 256] output column-slices, adds b_proj (exact for
the final linear step).
"""

import numpy as np

import concourse.bass as bass
import concourse.bacc as bacc
import concourse.mybir as mybir
import concourse.tile as tile
from concourse import bass_utils

F32 = mybir.dt.float32
F32R = mybir.dt.float32r
BF16 = mybir.dt.bfloat16

B, T, C = 2, 2048, 1024
H, D = 16, 64
DP, TP = 2, 4
HPC = H // TP            # 4 heads per core
CH = HPC * D             # 256 channels per core
NCORES = DP * TP

RG8 = [[0, 1, 2, 3, 4, 5, 6, 7]]


def build_kernel(trace_sim: bool = False):
    nc = bacc.Bacc("TRN2", target_bir_lowering=False, debug=False,
                   num_devices=NCORES)

    x_t = nc.dram_tensor("x_t", [C, T], F32R, kind="ExternalInput").ap()
    w_qk = nc.dram_tensor("w_qk", [C, 2 * CH], F32R, kind="ExternalInput").ap()
    b_qk = nc.dram_tensor("b_qk", [2 * CH], F32, kind="ExternalInput").ap()
    w_v = nc.dram_tensor("w_v", [C, CH], F32R, kind="ExternalInput").ap()
    b_v = nc.dram_tensor("b_v", [CH], F32, kind="ExternalInput").ap()
    w_p = nc.dram_tensor("w_p", [B * C, CH], F32, kind="ExternalInput").ap()
    out = nc.dram_tensor("out", [T, CH], F32, kind="ExternalOutput").ap()

    KT = C // 128        # 8 contraction tiles for C
    KT2 = B * C // 128   # 16 contraction tiles for the padded projection
    NTT = T // 128       # 16 token tiles
    NTC = T // 512       # 4 token chunks

    from contextlib import ExitStack
    with tile.TileContext(nc, trace_sim=trace_sim) as tc, ExitStack() as ctx:
        const = ctx.enter_context(tc.tile_pool(name="const", bufs=1))
        qkp = ctx.enter_context(tc.tile_pool(name="qkp", bufs=1))
        vp = ctx.enter_context(tc.tile_pool(name="vp", bufs=1))
        yp = ctx.enter_context(tc.tile_pool(name="yp", bufs=1))
        ep = ctx.enter_context(tc.tile_pool(name="ep", bufs=4))
        rp = ctx.enter_context(tc.tile_pool(name="rp", bufs=2))
        rbp = ctx.enter_context(tc.tile_pool(name="rbp", bufs=2))
        wpp = ctx.enter_context(tc.tile_pool(name="wpp", bufs=1))
        yfp = ctx.enter_context(tc.tile_pool(name="yfp", bufs=1))
        osb = ctx.enter_context(tc.tile_pool(name="osb", bufs=3))
        dram = ctx.enter_context(tc.tile_pool(name="dram", bufs=1, space="DRAM"))

        # ---- persistent SBUF tensors + loads -------------------------------
        Wqk = [const.tile([128, 2 * CH], F32R, name=f"wqk{k}") for k in range(KT)]
        Wv = [const.tile([128, CH], F32R, name=f"wv{k}") for k in range(KT)]
        bqk = const.tile([128, 4], F32, name="bqk")
        bvrow = const.tile([1, CH], F32, name="bvrow")
        bvbc = const.tile([128, CH], F32, name="bvbc")
        ones4 = const.tile([128, 4], F32, name="ones4")
        nc.vector.memset(ones4[:], 1.0)
        # causal masks for the two diagonal s-tile-pair positions, applied
        # multiplicatively on DVE (keeps GpSimd free so AllGather triggers
        # are never queued behind mask work)
        dmask = [const.tile([128, 1024], BF16, name=f"dmask{r}") for r in range(2)]
        for r in range(2):
            nc.gpsimd.memset(dmask[r][:], 1.0)
            nc.gpsimd.affine_select(
                out=dmask[r][:], in_=dmask[r][:],
                compare_op=mybir.AluOpType.is_ge, fill=0.0,
                base=-256 * r, pattern=[[-128, 2], [1, 512]],
                channel_multiplier=-1)

        cc_win = dram.tile([8, 16], BF16, name="cc_win")
        cc_wout = dram.tile([64, 16], BF16, name="cc_wout", addr_space="Shared")
        warm_sb = const.tile([8, 16], BF16, name="warm_sb")
        nc.vector.memset(warm_sb[:], 0.0)
        # dummy exp so the ~2.7us ACT table load happens during the DMA phase
        act_warm = const.tile([1, 16], F32, name="act_warm")
        nc.vector.memset(act_warm[:], 0.0)
        nc.scalar.activation(act_warm[:], act_warm[:],
                             mybir.ActivationFunctionType.Exp)
        nc.sync.dma_start(cc_win[:], warm_sb[:])
        nc.gpsimd.collective_compute(
            "AllGather", mybir.AluOpType.bypass, replica_groups=RG8,
            ins=[cc_win.opt()], outs=[cc_wout.opt()])
        for k in range(KT):
            nc.sync.dma_start(Wqk[k][:], w_qk[128 * k:128 * k + 128, :])
        nc.sync.dma_start(bqk[:], b_qk.rearrange("(i p) -> p i", p=128))
        nc.sync.dma_start(bvrow[:], b_v.unsqueeze(0))
        nc.gpsimd.partition_broadcast(bvbc[:], bvrow[:])

        # W_proj (padded to 2048 rows) -> bf16 tiles, converted on device.
        # (tiles declared here; DMAs emitted after the x loads below so the
        # first QKV matmuls aren't starved behind 2MB of projection weights)
        Wp = [wpp.tile([128, CH], BF16, name=f"wp{k}") for k in range(KT2)]
        wpf = [wpp.tile([128, CH], F32, name=f"wpf{k}") for k in range(KT2)]

        # qk^T tiles (bf16): [o-tile i][t-chunk j] -> [128, 512]
        # i = 0,1: q rows (pre-scaled by 1/sqrt(D) on host); i = 2,3: k rows
        qkT = [[qkp.tile([128, 512], BF16, name=f"qkT{i}_{j}") for j in range(NTC)]
               for i in range(4)]
        # v tiles (bf16), head-interleaved with a ones column: [128, 4*65]
        v_sb = [vp.tile([128, HPC * (D + 1)], BF16, name=f"v{m}") for m in range(NTT)]
        # normalized y^T chunk tiles (bf16): [chunk j] -> [256, 512] as 2x128
        yT = [[yp.tile([128, 512], BF16, name=f"yT{i}_{j}") for j in range(NTC)]
              for i in range(2)]

        # tiny warmup AllGather: pays the ncfw cold-start cost (~11us) during
        # the QKV phase so the first real AllGather begins promptly

        # AllGather bounce buffers: chunks 0+1 ship together (halves the
        # serialized collective count early on), chunks 2 and 3 ship alone
        cc_w = [1024, 512, 512]      # token width per ship s
        cc_in = [dram.tile([CH, w], BF16, name=f"cc_in{s}")
                 for s, w in enumerate(cc_w)]
        cc_out = [dram.tile([NCORES * CH, w], BF16, name=f"cc_out{s}",
                            addr_space="Shared")
                  for s, w in enumerate(cc_w)]

        xp = ctx.enter_context(tc.tile_pool(name="xp", bufs=1))
        # x^T loaded in 512-column chunks; chunk 0 lands right after Wqk so
        # the first QKV matmul group starts as early as possible
        xT = [[xp.tile([128, 512], F32R, name=f"xT{k}_{j}") for j in range(NTC)]
              for k in range(KT)]
        for k in range(KT):
            nc.sync.dma_start(xT[k][0][:],
                              x_t[128 * k:128 * k + 128, 0:512])
        for k in range(KT):
            nc.sync.dma_start(Wv[k][:], w_v[128 * k:128 * k + 128, :])
        for j in range(1, NTC):
            for k in range(KT):
                nc.sync.dma_start(xT[k][j][:],
                                  x_t[128 * k:128 * k + 128,
                                      512 * j:512 * j + 512])
        for k in range(KT2):
            nc.sync.dma_start(wpf[k][:], w_p[128 * k:128 * k + 128, :])
            nc.vector.tensor_copy(Wp[k][:], wpf[k][:])

        # ---- phases C/D/E: chunk-major attention + pipelined AG + proj -----
        def qk_group(j):
            # qk^T = W_qk^T @ x^T for one token chunk
            for i in range(4):
                ps = psM.tile([128, 512], F32, name="psA", tag="psM")
                for k in range(KT):
                    nc.tensor.matmul(
                        ps[:],
                        Wqk[k][:, 128 * i:128 * i + 128],
                        xT[k][j][:],
                        start=(k == 0), stop=(k == KT - 1))
                nc.vector.tensor_scalar_add(qkT[i][j][:], ps[:], bqk[:, i:i + 1])

        def v_group(j):
            # v = x @ W_v (natural layout, +bias, head-interleaved + ones col)
            for m in range(4 * j, 4 * j + 4):
                ones_ap = v_sb[m].rearrange("p (h x) -> p h x", x=D + 1)[:, :, D:D + 1]
                nc.vector.tensor_copy(ones_ap, ones4.rearrange("p (h x) -> p h x", x=1))
                ps = psM.tile([128, CH], F32, name="psB", tag="psM")
                for k in range(KT):
                    nc.tensor.matmul(
                        ps[:],
                        xT[k][m // 4][:, 128 * (m % 4):128 * (m % 4) + 128],
                        Wv[k][:],
                        start=(k == 0), stop=(k == KT - 1))
                v_ap = v_sb[m].rearrange("p (h x) -> p h x", x=D + 1)[:, :, 0:D]
                nc.vector.tensor_add(
                    v_ap,
                    ps.rearrange("p (h d) -> p h d", d=D),
                    bvbc.rearrange("p (h d) -> p h d", d=D))

        # Heads are processed in pairs (2hp, 2hp+1). The even head's q/k rows
        # live at partitions 0-63, the odd head's at 64-127, so interleaving
        # their S matmuls alternates PE row groups (tile_position auto-derives
        # from base_partition): the next weight load overlaps the in-flight
        # matmul and the two K=64 matmuls stream concurrently.
        def attn_chunk(j):
            for hp in range(HPC // 2):
                attn_pair(j, hp)

        def attn_pair(j, hp):
                ha, hb = 2 * hp, 2 * hp + 1
                y_psA = psY.tile([D + 1, 512], F32, name="y_psA", tag="y_ps")
                y_psB = psY.tile([D + 1, 512], F32, name="y_psB", tag="y_ps")
                n_s = 4 * (j + 1)           # causal s-tiles for this chunk
                for sp in range(n_s // 2):  # pairs of 128-row s-tiles
                    sA = psS.tile([128, 1024], F32, name="sA", tag="s_ps")
                    sB = psS.tile([128, 1024], F32, name="sB", tag="s_ps")
                    eA = ep.tile([128, 1024], BF16, name="eA", tag="e_sb")
                    eB = ep.tile([128, 1024], BF16, name="eB", tag="e_sb")
                    for half in range(2):
                        st = 2 * sp + half
                        kt = qkT[2 + hp][st // 4]
                        qt = qkT[hp][j]
                        ks = 128 * (st % 4)
                        nc.tensor.matmul(
                            sA[:, 512 * half:512 * half + 512],
                            kt[0:64, ks:ks + 128], qt[0:64, :],
                            start=True, stop=True)
                        nc.tensor.matmul(
                            sB[:, 512 * half:512 * half + 512],
                            kt[64:128, ks:ks + 128], qt[64:128, :],
                            start=True, stop=True)
                    nc.scalar.activation(
                        eA[:], sA[:], mybir.ActivationFunctionType.Exp)
                    nc.scalar.activation(
                        eB[:], sB[:], mybir.ActivationFunctionType.Exp)
                    if 2 * sp >= 4 * j:     # pair straddles the diagonal
                        r_idx = (2 * sp - 4 * j) // 2
                        for e in (eA, eB):
                            nc.vector.tensor_mul(e[:], e[:], dmask[r_idx][:])
                    for half in range(2):
                        st = 2 * sp + half
                        nc.tensor.matmul(
                            y_psA[:],
                            v_sb[st][:, (D + 1) * ha:(D + 1) * ha + D + 1],
                            eA[:, 512 * half:512 * half + 512],
                            start=(st == 0), stop=(st == n_s - 1))
                        nc.tensor.matmul(
                            y_psB[:],
                            v_sb[st][:, (D + 1) * hb:(D + 1) * hb + D + 1],
                            eB[:, 512 * half:512 * half + 512],
                            start=(st == 0), stop=(st == n_s - 1))
                # normalize: y * (1/rowsum); broadcast the raw rowsum across
                # partitions first so the reciprocal runs at full width
                for hh, y_ps in ((ha, y_psA), (hb, y_psB)):
                    r_sb = rp.tile([1, 512], F32, name="r_sb", tag="r_sb")
                    nc.vector.tensor_copy(r_sb[:], y_ps[D:D + 1, :])
                    rbc = rbp.tile([D, 512], F32, name="rbc", tag="rbc")
                    rinv = rbp.tile([D, 512], F32, name="rinv", tag="rinv")
                    nc.gpsimd.partition_broadcast(rbc[:], r_sb[:])
                    nc.vector.reciprocal_approx_fast(rinv[:], rbc[:])
                    nc.vector.tensor_mul(
                        yT[hp][j][64 * (hh % 2):64 * (hh % 2) + 64, :],
                        y_ps[0:D, :], rinv[:])

        def ship(s, chunks):
            for i in range(2):
                for ci, j in enumerate(chunks):
                    nc.sync.dma_start(
                        cc_in[s][128 * i:128 * i + 128,
                                 512 * ci:512 * ci + 512],
                        yT[i][j][:])
            nc.gpsimd.collective_compute(
                "AllGather", mybir.AluOpType.bypass,
                replica_groups=RG8,
                ins=[cc_in[s].opt()], outs=[cc_out[s].opt()])

        def load_yf(s):
            w = cc_w[s]
            yf = [yfp.tile([128, w], BF16, name=f"yf{k}", tag=f"yf{k}")
                  for k in range(KT2)]
            for k in range(KT2):
                nc.sync.dma_start(yf[k][:],
                                  cc_out[s][128 * k:128 * k + 128, :])
            return yf

        def proj_part(tok0, yf, ms):
            # ms indexes 128-token tiles within this ship's gathered width
            for m in ms:
                o_sb = osb.tile([128, CH], F32, name="o_sb", tag="o_sb")
                ps = psM.tile([128, CH], F32, name="psE", tag="psM")
                for k in range(KT2):
                    nc.tensor.matmul(
                        ps[:],
                        yf[k][:, 128 * m:128 * m + 128],
                        Wp[k][:],
                        start=(k == 0), stop=(k == KT2 - 1))
                nc.vector.tensor_copy(o_sb[:], ps[:])
                nc.scalar.dma_start(
                    out[tok0 + 128 * m:tok0 + 128 * m + 128, :],
                    o_sb[:])

        # QKV chunk-groups feed directly into their attention chunks: the
        # dense QKV/proj matmuls interleave with the ACT-bound attention so
        # the PE never idles long enough for HAM to re-throttle. Chunks 0+1
        # gather together; their projection rides inside attention chunk 3.
        with tc.tile_pool(name="psS", bufs=2, space="PSUM") as psS, \
             tc.tile_pool(name="psY", bufs=2, space="PSUM") as psY, \
             tc.tile_pool(name="psM", bufs=2, space="PSUM") as psM:
            qk_group(0)
            v_group(0)
            attn_chunk(0)
            qk_group(1)
            v_group(1)
            attn_chunk(1)
            ship(0, [0, 1])
            qk_group(2)
            v_group(2)
            attn_chunk(2)
            ship(1, [2])
            qk_group(3)
            v_group(3)
            yf01 = load_yf(0)
            attn_pair(3, 0)
            proj_part(0, yf01, [0, 1, 2, 3])
            attn_pair(3, 1)
            proj_part(0, yf01, [4, 5, 6, 7])
            ship(2, [3])
            proj_part(1024, load_yf(1), [0, 1, 2, 3])
            proj_part(1536, load_yf(2), [0, 1, 2, 3])

    nc.compile()
    return nc


def shard_inputs(x, W_attn, b_attn, W_proj, b_proj):
    scale = np.float32(D ** -0.5)
    in_maps = []
    for c in range(NCORES):
        b, g = divmod(c, TP)
        q = slice(CH * g, CH * (g + 1))
        k = slice(C + CH * g, C + CH * (g + 1))
        v = slice(2 * C + CH * g, 2 * C + CH * (g + 1))
        W_qk = np.concatenate([W_attn[:, q] * scale, W_attn[:, k]], axis=1)
        b_qk = np.concatenate([b_attn[q] * scale, b_attn[k]])
        # W_proj column slice, zero-padded to 2048 rows so the contraction
        # over the 8-core-gathered [2048, t] y picks out this core's batch
        w_p_pad = np.zeros((B * C, CH), dtype=np.float32)
        w_p_pad[C * b:C * (b + 1)] = W_proj[:, CH * g:CH * (g + 1)]
        in_maps.append({
            "x_t": np.ascontiguousarray(x[b].T, dtype=np.float32),
            "w_qk": np.ascontiguousarray(W_qk, dtype=np.float32),
            "b_qk": np.ascontiguousarray(b_qk, dtype=np.float32),
            "w_v": np.ascontiguousarray(W_attn[:, v], dtype=np.float32),
            "b_v": np.ascontiguousarray(b_attn[v], dtype=np.float32),
            "w_p": w_p_pad,
        })
    return in_maps


_NC_CACHE = {}


def get_compiled():
    if "nc" not in _NC_CACHE:
        _NC_CACHE["nc"] = build_kernel()
    return _NC_CACHE["nc"]


def run_on_hw(in_maps, **kwargs):
    nc = get_compiled()
    return bass_utils.run_bass_kernel_spmd(
        nc, in_maps, core_ids=list(range(NCORES)), **kwargs)


def kernel(x, W_attn, b_attn, W_proj, b_proj):
    x = np.asarray(x, dtype=np.float32)
    W_attn = np.asarray(W_attn, dtype=np.float32)
    b_attn = np.asarray(b_attn, dtype=np.float32)
    W_proj = np.asarray(W_proj, dtype=np.float32)
    b_proj = np.asarray(b_proj, dtype=np.float32)

    in_maps = shard_inputs(x, W_attn, b_attn, W_proj, b_proj)
    res = run_on_hw(in_maps)
    out = np.zeros((B, T, C), dtype=np.float32)
    for c in range(NCORES):
        b, g = divmod(c, TP)
        out[b, :, CH * g:CH * (g + 1)] = res.results[c]["out"]
    out += b_proj[None, None, :]
    return out


# revision 36
# speedup vs baseline: 1.0817x; 1.0046x over previous
"""Distributed causal self-attention kernel for one TRN2 chip (8 NeuronCores).

Problem: B=2, T=2048, C=1024, H=16 heads, D=64. f32 in/out.
Measured: ~252 us NEFF exec time, rel err (fro) ~3.8e-3 vs the fp32 reference.

Sharding: DP=2 over batch x TP=4 over heads.
  core c -> (b = c//4, g = c%4), owns heads 4g..4g+3 of batch b.

Per-core device program (SPMD, identical graph on all 8 cores), built with
Tile and scheduled as one fused stream so the PE never idles long enough for
the HAM clock gate to re-throttle:

  - startup-critical loads (W_qk + the first x^T chunk) are issued half on
    the Sync and half on the Scalar sequencer: every dma_start costs ~0.65us
    of serial issue time on its engine, so splitting halves the time to the
    first matmul.
  - qk^T = (x @ [W_q/8 | W_k])^T computed directly in transposed layout via
    matmul(lhsT=W_qk_tile, rhs=x^T_tile); x^T is fed pre-transposed from the
    host, so NO on-chip transposes are needed anywhere. float32r inputs
    (full-rate fp32 PE path), fp32 PSUM, bf16 evacuation fused with the
    per-partition q/k bias add.
  - v = x @ W_v in natural layout, head-interleaved with a ones column
    -> lhsT = [v_h | 1] so the attention AV matmul also produces the softmax
    row-sums for free (row 64 of the accumulator).
  - attention per 512-token chunk, heads in pairs: the even head's q/k rows
    sit at partitions 0-63 and the odd head's at 64-127, so interleaved S^T
    matmuls (K=64, bf16) alternate PE row groups and their weight loads
    overlap in-flight matmuls. exp on ScalarE ([128,1024] PSUM->SBUF, bf16
    out, softmax scale pre-folded into W_q on the host); causal masking of
    diagonal tiles via precomputed multiplicative bf16 masks on VectorE
    (keeps GpSimd free so collective triggers fire promptly). No max-
    subtraction is needed: S = qk/sqrt(D) is O(5) here, exp is safe in fp32.
    Normalization: rowsum broadcast across partitions (GpSimd), fast
    reciprocal + multiply on VectorE, bf16 y^T.
  - cross-core reduction: per-chunk 8-core AllGather of bf16 y^T chunks
    (chunks 0+1 merged; a tiny warmup AllGather at kernel start hides the
    ncfw cold-start). Output rows are rank-major = [batch0 | batch1]
    channels, so every core reads identical offsets (SPMD-safe); the host
    zero-pads each core's W_proj column-slice to 2048 rows so the projection
    contraction picks out its own batch.
  - projection out^chunk = y_gath^T.T @ W_proj_pad (bf16), interleaved into
    later attention chunks' head pairs to fill ACT-bound PE bubbles and to
    guarantee each gather has completed before its matmuls issue (the PE
    stream is in-order).

Host: shards inputs (x transposed per batch, W_attn column-sliced with the
softmax scale folded into W_q, W_proj column-sliced + batch-zero-padded),
reassembles the 8 [2048, 256] output column-slices, adds b_proj (exact for
the final linear step).
"""

import numpy as np

import concourse.bass as bass
import concourse.bacc as bacc
import concourse.mybir as mybir
import concourse.tile as tile
from concourse import bass_utils

F32 = mybir.dt.float32
F32R = mybir.dt.float32r
BF16 = mybir.dt.bfloat16

B, T, C = 2, 2048, 1024
H, D = 16, 64
DP, TP = 2, 4
HPC = H // TP            # 4 heads per core
CH = HPC * D             # 256 channels per core
NCORES = DP * TP

RG8 = [[0, 1, 2, 3, 4, 5, 6, 7]]


def build_kernel(trace_sim: bool = False):
    nc = bacc.Bacc("TRN2", target_bir_lowering=False, debug=False,
                   num_devices=NCORES)

    x_t = nc.dram_tensor("x_t", [C, T], F32R, kind="ExternalInput").ap()
    w_qk = nc.dram_tensor("w_qk", [C, 2 * CH], F32R, kind="ExternalInput").ap()
    b_qk = nc.dram_tensor("b_qk", [2 * CH], F32, kind="ExternalInput").ap()
    w_v = nc.dram_tensor("w_v", [C, CH], F32R, kind="ExternalInput").ap()
    b_v = nc.dram_tensor("b_v", [CH], F32, kind="ExternalInput").ap()
    w_p = nc.dram_tensor("w_p", [B * C, CH], F32, kind="ExternalInput").ap()
    out = nc.dram_tensor("out", [T, CH], F32, kind="ExternalOutput").ap()

    KT = C // 128        # 8 contraction tiles for C
    KT2 = B * C // 128   # 16 contraction tiles for the padded projection
    NTT = T // 128       # 16 token tiles
    NTC = T // 512       # 4 token chunks

    from contextlib import ExitStack
    with tile.TileContext(nc, trace_sim=trace_sim) as tc, ExitStack() as ctx:
        const = ctx.enter_context(tc.tile_pool(name="const", bufs=1))
        qkp = ctx.enter_context(tc.tile_pool(name="qkp", bufs=1))
        vp = ctx.enter_context(tc.tile_pool(name="vp", bufs=1))
        yp = ctx.enter_context(tc.tile_pool(name="yp", bufs=1))
        ep = ctx.enter_context(tc.tile_pool(name="ep", bufs=4))
        rp = ctx.enter_context(tc.tile_pool(name="rp", bufs=2))
        rbp = ctx.enter_context(tc.tile_pool(name="rbp", bufs=2))
        wpp = ctx.enter_context(tc.tile_pool(name="wpp", bufs=1))
        yfp = ctx.enter_context(tc.tile_pool(name="yfp", bufs=1))
        osb = ctx.enter_context(tc.tile_pool(name="osb", bufs=3))
        dram = ctx.enter_context(tc.tile_pool(name="dram", bufs=1, space="DRAM"))

        # ---- persistent SBUF tensors + loads -------------------------------
        Wqk = [const.tile([128, 2 * CH], F32R, name=f"wqk{k}") for k in range(KT)]
        Wv = [const.tile([128, CH], F32R, name=f"wv{k}") for k in range(KT)]
        bqk = const.tile([128, 4], F32, name="bqk")
        bvrow = const.tile([1, CH], F32, name="bvrow")
        bvbc = const.tile([128, CH], F32, name="bvbc")
        ones4 = const.tile([128, 4], F32, name="ones4")
        nc.vector.memset(ones4[:], 1.0)
        # causal masks for the two diagonal s-tile-pair positions, applied
        # multiplicatively on DVE (keeps GpSimd free so AllGather triggers
        # are never queued behind mask work)
        dmask = [const.tile([128, 1024], BF16, name=f"dmask{r}") for r in range(2)]
        for r in range(2):
            nc.gpsimd.memset(dmask[r][:], 1.0)
            nc.gpsimd.affine_select(
                out=dmask[r][:], in_=dmask[r][:],
                compare_op=mybir.AluOpType.is_ge, fill=0.0,
                base=-256 * r, pattern=[[-128, 2], [1, 512]],
                channel_multiplier=-1)

        cc_win = dram.tile([8, 16], BF16, name="cc_win")
        cc_wout = dram.tile([64, 16], BF16, name="cc_wout", addr_space="Shared")
        warm_sb = const.tile([8, 16], BF16, name="warm_sb")
        nc.vector.memset(warm_sb[:], 0.0)
        # dummy exp so the ~2.7us ACT table load happens during the DMA phase
        act_warm = const.tile([1, 16], F32, name="act_warm")
        nc.vector.memset(act_warm[:], 0.0)
        nc.scalar.activation(act_warm[:], act_warm[:],
                             mybir.ActivationFunctionType.Exp)
        nc.sync.dma_start(cc_win[:], warm_sb[:])
        nc.gpsimd.collective_compute(
            "AllGather", mybir.AluOpType.bypass, replica_groups=RG8,
            ins=[cc_win.opt()], outs=[cc_wout.opt()])
        for k in range(KT):
            eng = nc.sync if k < 4 else nc.scalar
            eng.dma_start(Wqk[k][:], w_qk[128 * k:128 * k + 128, :])
        nc.sync.dma_start(bqk[:], b_qk.rearrange("(i p) -> p i", p=128))
        nc.sync.dma_start(bvrow[:], b_v.unsqueeze(0))
        nc.gpsimd.partition_broadcast(bvbc[:], bvrow[:])

        # W_proj (padded to 2048 rows) -> bf16 tiles, converted on device.
        # (tiles declared here; DMAs emitted after the x loads below so the
        # first QKV matmuls aren't starved behind 2MB of projection weights)
        Wp = [wpp.tile([128, CH], BF16, name=f"wp{k}") for k in range(KT2)]
        wpf = [wpp.tile([128, CH], F32, name=f"wpf{k}") for k in range(KT2)]

        # qk^T tiles (bf16): [o-tile i][t-chunk j] -> [128, 512]
        # i = 0,1: q rows (pre-scaled by 1/sqrt(D) on host); i = 2,3: k rows
        qkT = [[qkp.tile([128, 512], BF16, name=f"qkT{i}_{j}") for j in range(NTC)]
               for i in range(4)]
        # v tiles (bf16), head-interleaved with a ones column: [128, 4*65]
        v_sb = [vp.tile([128, HPC * (D + 1)], BF16, name=f"v{m}") for m in range(NTT)]
        # normalized y^T chunk tiles (bf16): [chunk j] -> [256, 512] as 2x128
        yT = [[yp.tile([128, 512], BF16, name=f"yT{i}_{j}") for j in range(NTC)]
              for i in range(2)]

        # tiny warmup AllGather: pays the ncfw cold-start cost (~11us) during
        # the QKV phase so the first real AllGather begins promptly

        # AllGather bounce buffers: chunks 0+1 ship together (halves the
        # serialized collective count early on), chunks 2 and 3 ship alone
        cc_w = [1024, 512, 512]      # token width per ship s
        cc_in = [dram.tile([CH, w], BF16, name=f"cc_in{s}")
                 for s, w in enumerate(cc_w)]
        cc_out = [dram.tile([NCORES * CH, w], BF16, name=f"cc_out{s}",
                            addr_space="Shared")
                  for s, w in enumerate(cc_w)]

        xp = ctx.enter_context(tc.tile_pool(name="xp", bufs=1))
        # x^T loaded in 512-column chunks; chunk 0 lands right after Wqk so
        # the first QKV matmul group starts as early as possible
        xT = [[xp.tile([128, 512], F32R, name=f"xT{k}_{j}") for j in range(NTC)]
              for k in range(KT)]
        for k in range(KT):
            eng = nc.sync if k < 4 else nc.scalar
            eng.dma_start(xT[k][0][:],
                          x_t[128 * k:128 * k + 128, 0:512])
        for k in range(KT):
            nc.sync.dma_start(Wv[k][:], w_v[128 * k:128 * k + 128, :])
        for j in range(1, NTC):
            for k in range(KT):
                nc.sync.dma_start(xT[k][j][:],
                                  x_t[128 * k:128 * k + 128,
                                      512 * j:512 * j + 512])
        for k in range(KT2):
            nc.sync.dma_start(wpf[k][:], w_p[128 * k:128 * k + 128, :])
            nc.vector.tensor_copy(Wp[k][:], wpf[k][:])

        # ---- phases C/D/E: chunk-major attention + pipelined AG + proj -----
        def qk_group(j):
            # qk^T = W_qk^T @ x^T for one token chunk
            for i in range(4):
                ps = psM.tile([128, 512], F32, name="psA", tag="psM")
                for k in range(KT):
                    nc.tensor.matmul(
                        ps[:],
                        Wqk[k][:, 128 * i:128 * i + 128],
                        xT[k][j][:],
                        start=(k == 0), stop=(k == KT - 1))
                nc.vector.tensor_scalar_add(qkT[i][j][:], ps[:], bqk[:, i:i + 1])

        def v_group(j):
            # v = x @ W_v (natural layout, +bias, head-interleaved + ones col)
            for m in range(4 * j, 4 * j + 4):
                ones_ap = v_sb[m].rearrange("p (h x) -> p h x", x=D + 1)[:, :, D:D + 1]
                nc.vector.tensor_copy(ones_ap, ones4.rearrange("p (h x) -> p h x", x=1))
                ps = psM.tile([128, CH], F32, name="psB", tag="psM")
                for k in range(KT):
                    nc.tensor.matmul(
                        ps[:],
                        xT[k][m // 4][:, 128 * (m % 4):128 * (m % 4) + 128],
                        Wv[k][:],
                        start=(k == 0), stop=(k == KT - 1))
                v_ap = v_sb[m].rearrange("p (h x) -> p h x", x=D + 1)[:, :, 0:D]
                nc.vector.tensor_add(
                    v_ap,
                    ps.rearrange("p (h d) -> p h d", d=D),
                    bvbc.rearrange("p (h d) -> p h d", d=D))

        # Heads are processed in pairs (2hp, 2hp+1). The even head's q/k rows
        # live at partitions 0-63, the odd head's at 64-127, so interleaving
        # their S matmuls alternates PE row groups (tile_position auto-derives
        # from base_partition): the next weight load overlaps the in-flight
        # matmul and the two K=64 matmuls stream concurrently.
        def attn_chunk(j):
            for hp in range(HPC // 2):
                attn_pair(j, hp)

        def attn_pair(j, hp):
                ha, hb = 2 * hp, 2 * hp + 1
                y_psA = psY.tile([D + 1, 512], F32, name="y_psA", tag="y_ps")
                y_psB = psY.tile([D + 1, 512], F32, name="y_psB", tag="y_ps")
                n_s = 4 * (j + 1)           # causal s-tiles for this chunk
                for sp in range(n_s // 2):  # pairs of 128-row s-tiles
                    sA = psS.tile([128, 1024], F32, name="sA", tag="s_ps")
                    sB = psS.tile([128, 1024], F32, name="sB", tag="s_ps")
                    eA = ep.tile([128, 1024], BF16, name="eA", tag="e_sb")
                    eB = ep.tile([128, 1024], BF16, name="eB", tag="e_sb")
                    for half in range(2):
                        st = 2 * sp + half
                        kt = qkT[2 + hp][st // 4]
                        qt = qkT[hp][j]
                        ks = 128 * (st % 4)
                        nc.tensor.matmul(
                            sA[:, 512 * half:512 * half + 512],
                            kt[0:64, ks:ks + 128], qt[0:64, :],
                            start=True, stop=True)
                        nc.tensor.matmul(
                            sB[:, 512 * half:512 * half + 512],
                            kt[64:128, ks:ks + 128], qt[64:128, :],
                            start=True, stop=True)
                    nc.scalar.activation(
                        eA[:], sA[:], mybir.ActivationFunctionType.Exp)
                    nc.scalar.activation(
                        eB[:], sB[:], mybir.ActivationFunctionType.Exp)
                    if 2 * sp >= 4 * j:     # pair straddles the diagonal
                        r_idx = (2 * sp - 4 * j) // 2
                        for e in (eA, eB):
                            nc.vector.tensor_mul(e[:], e[:], dmask[r_idx][:])
                    for half in range(2):
                        st = 2 * sp + half
                        nc.tensor.matmul(
                            y_psA[:],
                            v_sb[st][:, (D + 1) * ha:(D + 1) * ha + D + 1],
                            eA[:, 512 * half:512 * half + 512],
                            start=(st == 0), stop=(st == n_s - 1))
                        nc.tensor.matmul(
                            y_psB[:],
                            v_sb[st][:, (D + 1) * hb:(D + 1) * hb + D + 1],
                            eB[:, 512 * half:512 * half + 512],
                            start=(st == 0), stop=(st == n_s - 1))
                # normalize: y * (1/rowsum); broadcast the raw rowsum across
                # partitions first so the reciprocal runs at full width
                for hh, y_ps in ((ha, y_psA), (hb, y_psB)):
                    r_sb = rp.tile([1, 512], F32, name="r_sb", tag="r_sb")
                    nc.vector.tensor_copy(r_sb[:], y_ps[D:D + 1, :])
                    rbc = rbp.tile([D, 512], F32, name="rbc", tag="rbc")
                    rinv = rbp.tile([D, 512], F32, name="rinv", tag="rinv")
                    nc.gpsimd.partition_broadcast(rbc[:], r_sb[:])
                    nc.vector.reciprocal_approx_fast(rinv[:], rbc[:])
                    nc.vector.tensor_mul(
                        yT[hp][j][64 * (hh % 2):64 * (hh % 2) + 64, :],
                        y_ps[0:D, :], rinv[:])

        def ship(s, chunks):
            for i in range(2):
                for ci, j in enumerate(chunks):
                    nc.sync.dma_start(
                        cc_in[s][128 * i:128 * i + 128,
                                 512 * ci:512 * ci + 512],
                        yT[i][j][:])
            nc.gpsimd.collective_compute(
                "AllGather", mybir.AluOpType.bypass,
                replica_groups=RG8,
                ins=[cc_in[s].opt()], outs=[cc_out[s].opt()])

        def load_yf(s):
            w = cc_w[s]
            yf = [yfp.tile([128, w], BF16, name=f"yf{k}", tag=f"yf{k}")
                  for k in range(KT2)]
            for k in range(KT2):
                nc.sync.dma_start(yf[k][:],
                                  cc_out[s][128 * k:128 * k + 128, :])
            return yf

        def proj_part(tok0, yf, ms):
            # ms indexes 128-token tiles within this ship's gathered width
            for m in ms:
                o_sb = osb.tile([128, CH], F32, name="o_sb", tag="o_sb")
                ps = psM.tile([128, CH], F32, name="psE", tag="psM")
                for k in range(KT2):
                    nc.tensor.matmul(
                        ps[:],
                        yf[k][:, 128 * m:128 * m + 128],
                        Wp[k][:],
                        start=(k == 0), stop=(k == KT2 - 1))
                nc.vector.tensor_copy(o_sb[:], ps[:])
                nc.scalar.dma_start(
                    out[tok0 + 128 * m:tok0 + 128 * m + 128, :],
                    o_sb[:])

        # QKV chunk-groups feed directly into their attention chunks: the
        # dense QKV/proj matmuls interleave with the ACT-bound attention so
        # the PE never idles long enough for HAM to re-throttle. Chunks 0+1
        # gather together; their projection rides inside attention chunk 3.
        with tc.tile_pool(name="psS", bufs=2, space="PSUM") as psS, \
             tc.tile_pool(name="psY", bufs=2, space="PSUM") as psY, \
             tc.tile_pool(name="psM", bufs=2, space="PSUM") as psM:
            qk_group(0)
            v_group(0)
            attn_chunk(0)
            qk_group(1)
            v_group(1)
            attn_chunk(1)
            ship(0, [0, 1])
            qk_group(2)
            v_group(2)
            attn_chunk(2)
            ship(1, [2])
            qk_group(3)
            v_group(3)
            yf01 = load_yf(0)
            attn_pair(3, 0)
            proj_part(0, yf01, [0, 1, 2, 3])
            attn_pair(3, 1)
            proj_part(0, yf01, [4, 5, 6, 7])
            ship(2, [3])
            proj_part(1024, load_yf(1), [0, 1, 2, 3])
            proj_part(1536, load_yf(2), [0, 1, 2, 3])

    nc.compile()
    return nc


def shard_inputs(x, W_attn, b_attn, W_proj, b_proj):
    scale = np.float32(D ** -0.5)
    in_maps = []
    for c in range(NCORES):
        b, g = divmod(c, TP)
        q = slice(CH * g, CH * (g + 1))
        k = slice(C + CH * g, C + CH * (g + 1))
        v = slice(2 * C + CH * g, 2 * C + CH * (g + 1))
        W_qk = np.concatenate([W_attn[:, q] * scale, W_attn[:, k]], axis=1)
        b_qk = np.concatenate([b_attn[q] * scale, b_attn[k]])
        # W_proj column slice, zero-padded to 2048 rows so the contraction
        # over the 8-core-gathered [2048, t] y picks out this core's batch
        w_p_pad = np.zeros((B * C, CH), dtype=np.float32)
        w_p_pad[C * b:C * (b + 1)] = W_proj[:, CH * g:CH * (g + 1)]
        in_maps.append({
            "x_t": np.ascontiguousarray(x[b].T, dtype=np.float32),
            "w_qk": np.ascontiguousarray(W_qk, dtype=np.float32),
            "b_qk": np.ascontiguousarray(b_qk, dtype=np.float32),
            "w_v": np.ascontiguousarray(W_attn[:, v], dtype=np.float32),
            "b_v": np.ascontiguousarray(b_attn[v], dtype=np.float32),
            "w_p": w_p_pad,
        })
    return in_maps


_NC_CACHE = {}


def get_compiled():
    if "nc" not in _NC_CACHE:
        _NC_CACHE["nc"] = build_kernel()
    return _NC_CACHE["nc"]


def run_on_hw(in_maps, **kwargs):
    nc = get_compiled()
    return bass_utils.run_bass_kernel_spmd(
        nc, in_maps, core_ids=list(range(NCORES)), **kwargs)


def kernel(x, W_attn, b_attn, W_proj, b_proj):
    x = np.asarray(x, dtype=np.float32)
    W_attn = np.asarray(W_attn, dtype=np.float32)
    b_attn = np.asarray(b_attn, dtype=np.float32)
    W_proj = np.asarray(W_proj, dtype=np.float32)
    b_proj = np.asarray(b_proj, dtype=np.float32)

    in_maps = shard_inputs(x, W_attn, b_attn, W_proj, b_proj)
    res = run_on_hw(in_maps)
    out = np.zeros((B, T, C), dtype=np.float32)
    for c in range(NCORES):
        b, g = divmod(c, TP)
        out[b, :, CH * g:CH * (g + 1)] = res.results[c]["out"]
    out += b_proj[None, None, :]
    return out
